# revision 10
# baseline (speedup 1.0000x reference)
"""CrossBlock (sine pos-emb + linear elu+1 attention + MLP) on 8 trn2 cores.

Sharding: tokens of each batch element (V*HW = 24005) split over 4 cores
(cores 0-3 = batch 0, cores 4-7 = batch 1), padded to R = 6144 per core.
Two SPMD launches: phase 1 computes per-shard partial kv = sum_l k v^T and
ksum = sum_l k (33 KB); the host reduces those across each batch's 4 cores;
phase 2 computes q, y = (q @ kv) * z, proj, and the MLP.  Everything runs in
[channel, token] (transposed) layout so no on-chip transposes are needed:
the host supplies x^T shards and transposes the output back.
"""
import sys, os, json, math
sys.path.insert(0, '/opt/trn_rl_repo')
import numpy as np

import concourse.bass as bass
import concourse.mybir as mybir
import concourse.tile as tile
from concourse.bass_utils import run_bass_kernel_spmd

FP32 = mybir.dt.float32
F32R = mybir.dt.float32r
BF16 = mybir.dt.bfloat16
ACT = mybir.ActivationFunctionType
ALU = mybir.AluOpType

B, V, Hh, Ww, C, NH = 2, 5, 60, 80, 256, 8
HW = Hh * Ww + 1
L = V * HW          # 24005 tokens per batch element
R = 6144            # tokens per core (padded); 4 cores per batch
T = 512             # token tile
NT = R // T
NCHUNK = T // 128   # 128-token chunks per tile
EPS = 1e-6
MAGIC = 12582912.0  # 1.5 * 2^23 fp32 round-to-nearest trick
TWO_PI = 2.0 * math.pi

# ---------------------------------------------------------------- bir fix --
def _fix_inst_list(lst, counter):
    out = []
    for ins in lst:
        if not (isinstance(ins, dict) and 'opcode' in ins and 'sync_info' in ins):
            out.append(ins); continue
        si = ins.get('sync_info') or {}
        waits = si.get('on_wait') or []
        ups = si.get('on_update') or []
        if len(waits) > 1:
            for w in waits[:-1]:
                counter[0] += 1
                out.append({"debug": ins.get("debug", 0), "engine": ins["engine"],
                            "ins": [], "outs": [], "name": f"I-wfix{counter[0]}",
                            "opcode": "EventSemaphore",
                            "sync_info": {"on_update": [], "on_wait": [w]}})
            si['on_wait'] = [waits[-1]]
        out.append(ins)
        if len(ups) > 1:
            si['on_update'] = [ups[0]]
            for u in ups[1:]:
                counter[0] += 1
                out.append({"debug": ins.get("debug", 0), "engine": ins["engine"],
                            "ins": [], "outs": [], "name": f"I-ufix{counter[0]}",
                            "opcode": "EventSemaphore",
                            "sync_info": {"on_update": [u], "on_wait": []}})
    return out


def _walk(o, counter):
    if isinstance(o, dict):
        for k, v in o.items():
            if isinstance(v, list) and v and isinstance(v[0], dict) and 'opcode' in v[0]:
                o[k] = _fix_inst_list(v, counter)
                for ins in o[k]:
                    _walk(ins, counter)
            else:
                _walk(v, counter)
    elif isinstance(o, list):
        for v in o:
            _walk(v, counter)


def _install_bir_fix():
    if getattr(bass.Bass, '_birfix_installed', False):
        return
    orig = bass.Bass.to_json_bytes

    def patched(self):
        m = json.loads(orig(self))
        _walk(m, [0])
        return json.dumps(m).encode()

    bass.Bass.to_json_bytes = patched
    bass.Bass._birfix_installed = True


_install_bir_fix()

# ------------------------------------------------------------- emit shared --
def _round_tile(nc, pool, name, src_dram, shape, dt=F32R):
    """DMA a small dram tensor to SBUF and produce a rounded (f32r) copy."""
    stg = pool.tile(shape, FP32, tag=f"stg_{name}")
    nc.sync.dma_start(stg[:], src_dram[:])
    r = pool.tile(shape, dt, tag=f"r_{name}")
    nc.vector.tensor_copy(r[:], stg[:])
    return r


def _emit_x1(nc, tc, io, consts, work, psum, i):
    """Emit x1^T = x^T + tok_emb^T for token tile i; returns (x1_0, x1_1) f32r."""
    sl = bass.ts(i, T)
    xt0 = work.tile([128, T], FP32, tag="xt0")
    nc.sync.dma_start(xt0[:], io['xT'][0:128, sl])
    xt1 = work.tile([128, T], FP32, tag="xt1")
    nc.sync.dma_start(xt1[:], io['xT'][128:256, sl])
    rel = work.tile([3, T], FP32, tag="rel")
    nc.sync.dma_start(rel[:], io['rel'][:, sl])
    sel = work.tile([6, T], FP32, tag="sel")
    nc.sync.dma_start(sel[:], io['sel'][:, sl])
    rel_r = work.tile([3, T], F32R, tag="rel_r")
    nc.vector.tensor_copy(rel_r[:], rel[:])
    sel_r = work.tile([6, T], F32R, tag="sel_r")
    nc.vector.tensor_copy(sel_r[:], sel[:])

    # phase matrix P = F.T @ [rel_v; rel_u; mask]  -> [128, T] (channels 128:256)
    ph = psum.tile([128, T], FP32, tag="ps")
    nc.tensor.matmul(ph[:], consts['F'][:], rel_r[:], start=True, stop=True)
    # range-reduce: x' = P - 2pi*round(P/2pi)   (ACT for the two scalar steps)
    t1 = work.tile([128, T], FP32, tag="sr1")
    nc.vector.tensor_scalar(t1[:], ph[:], 1.0 / TWO_PI, MAGIC, ALU.mult, ALU.add)
    t2 = work.tile([128, T], FP32, tag="sr2")
    nc.vector.tensor_scalar(t2[:], t1[:], MAGIC, -TWO_PI, ALU.subtract, ALU.mult)
    t3 = work.tile([128, T], FP32, tag="sr3")
    nc.vector.tensor_tensor(t3[:], t2[:], ph[:], ALU.add)
    sinp = work.tile([128, T], FP32, tag="sinp")
    nc.scalar.activation(sinp[:], t3[:], ACT.Sin)

    # const part (two 128-channel chunks) from table
    c0 = psum.tile([128, T], FP32, tag="ps")
    nc.tensor.matmul(c0[:], consts['tbl'][:, 0:128], sel_r[:], start=True, stop=True)
    c1 = psum.tile([128, T], FP32, tag="ps")
    nc.tensor.matmul(c1[:], consts['tbl'][:, 128:256], sel_r[:], start=True, stop=True)

    x1_0 = work.tile([128, T], F32R, tag="x1_0")
    nc.vector.tensor_tensor(x1_0[:], xt0[:], c0[:], ALU.add)
    tmp = work.tile([128, T], FP32, tag="x1tmp")
    nc.vector.tensor_tensor(tmp[:], xt1[:], c1[:], ALU.add)
    x1_1 = work.tile([128, T], F32R, tag="x1_1")
    nc.vector.tensor_tensor(x1_1[:], tmp[:], sinp[:], ALU.add)
    return xt0, xt1, x1_0, x1_1


# --------------------------------------------------------------- phase 1 --
def build_phase1():
    nc = bass.Bass()
    io = {
        'xT': nc.dram_tensor("xT", [C, R], FP32, kind="ExternalInput"),
        'rel': nc.dram_tensor("rel", [3, R], FP32, kind="ExternalInput"),
        'sel': nc.dram_tensor("sel", [6, R], FP32, kind="ExternalInput"),
        'F': nc.dram_tensor("F", [3, 128], FP32, kind="ExternalInput"),
        'tbl': nc.dram_tensor("tbl", [6, C], FP32, kind="ExternalInput"),
        'w_kv': nc.dram_tensor("w_kv", [C, 2 * C], FP32, kind="ExternalInput"),
    }
    kv_outA = nc.dram_tensor("kvA", [128, C], FP32, kind="ExternalOutput")
    kv_outB = nc.dram_tensor("kvB", [128, C], FP32, kind="ExternalOutput")
    ks_out = nc.dram_tensor("ks", [1, C], FP32, kind="ExternalOutput")

    with nc.allow_low_precision(reason="bf16 kv accumulation is intended"), \
         tile.TileContext(nc) as tc:
        with tc.tile_pool(name="const", bufs=1) as cpool, \
             tc.tile_pool(name="work", bufs=3) as work, \
             tc.tile_pool(name="acc", bufs=1, space="PSUM") as accp, \
             tc.tile_pool(name="psum", bufs=4, space="PSUM") as psum:
            consts = {
                'F': _round_tile(nc, cpool, "F", io['F'], [3, 128]),
                'tbl': _round_tile(nc, cpool, "tbl", io['tbl'], [6, C]),
                'wkv0': _round_tile(nc, cpool, "wkv0", io['w_kv'][0:128, :], [128, 2 * C]),
                'wkv1': _round_tile(nc, cpool, "wkv1", io['w_kv'][128:256, :], [128, 2 * C]),
            }
            ones_s = cpool.tile([128, 1], FP32)
            nc.vector.memset(ones_s[:], 1.0)
            ones = cpool.tile([128, 1], F32R)
            nc.vector.tensor_copy(ones[:], ones_s[:])
            pkvA = accp.tile([128, C], FP32)
            pkvB = accp.tile([128, C], FP32)
            pks = accp.tile([1, C], FP32)

            nmm = NT * NCHUNK
            mm = 0
            for i in range(NT):
                _, _, x1_0, x1_1 = _emit_x1(nc, tc, io, consts, work, psum, i)
                for cch in range(NCHUNK):
                    csl = bass.ts(cch, 128)
                    # natural-layout k|v for these 128 tokens: [tok, 512]
                    kvn = psum.tile([128, 2 * C], FP32, tag="ps")
                    nc.tensor.matmul(kvn[:], x1_0[:, csl], consts['wkv0'][:],
                                     start=True, stop=False)
                    nc.tensor.matmul(kvn[:], x1_1[:, csl], consts['wkv1'][:],
                                     start=False, stop=True)
                    # k = elu(.)+1 in bf16; v plain bf16
                    r1 = work.tile([128, C], FP32, tag="r1")
                    nc.scalar.activation(r1[:], kvn[:, 0:C], ACT.Relu, scale=-1.0)
                    e1 = work.tile([128, C], FP32, tag="e1")
                    nc.scalar.activation(e1[:], r1[:], ACT.Exp, scale=-1.0)
                    k_bf = work.tile([128, C], F32R, tag="k_bf")
                    nc.vector.scalar_tensor_tensor(k_bf[:], kvn[:, 0:C], 0.0, e1[:],
                                                   ALU.max, ALU.add)
                    v_bf = work.tile([128, C], F32R, tag="v_bf")
                    nc.vector.tensor_copy(v_bf[:], kvn[:, C:2 * C])
                    first, last = mm == 0, mm == nmm - 1
                    nc.tensor.matmul(pkvA[:], v_bf[:, 0:128], k_bf[:],
                                     start=first, stop=last)
                    nc.tensor.matmul(pkvB[:], v_bf[:, 128:256], k_bf[:],
                                     start=first, stop=last)
                    nc.tensor.matmul(pks[:], ones[:], k_bf[:], start=first, stop=last)
                    mm += 1
            okvA = cpool.tile([128, C], FP32, tag="okvA")
            nc.vector.tensor_copy(okvA[:], pkvA[:])
            nc.sync.dma_start(kv_outA[:], okvA[:])
            okvB = cpool.tile([128, C], FP32, tag="okvB")
            nc.vector.tensor_copy(okvB[:], pkvB[:])
            nc.sync.dma_start(kv_outB[:], okvB[:])
            oks = cpool.tile([1, C], FP32, tag="oks")
            nc.vector.tensor_copy(oks[:], pks[:])
            nc.sync.dma_start(ks_out[:], oks[:])
    nc.finalize()
    return nc


# --------------------------------------------------------------- phase 2 --
def build_phase2():
    nc = bass.Bass()
    io = {
        'xT': nc.dram_tensor("xT", [C, R], FP32, kind="ExternalInput"),
        'rel': nc.dram_tensor("rel", [3, R], FP32, kind="ExternalInput"),
        'sel': nc.dram_tensor("sel", [6, R], FP32, kind="ExternalInput"),
        'F': nc.dram_tensor("F", [3, 128], FP32, kind="ExternalInput"),
        'tbl': nc.dram_tensor("tbl", [6, C], FP32, kind="ExternalInput"),
        'w_q': nc.dram_tensor("w_q", [C, C], FP32, kind="ExternalInput"),
        'w_proj': nc.dram_tensor("w_proj", [C, C], FP32, kind="ExternalInput"),
        'w_fc1': nc.dram_tensor("w_fc1", [C, 2 * C], FP32, kind="ExternalInput"),
        'w_fc2': nc.dram_tensor("w_fc2", [2 * C, C], FP32, kind="ExternalInput"),
        'kvd': nc.dram_tensor("kvd", [128, 2 * 128], FP32, kind="ExternalInput"),
        'ksd': nc.dram_tensor("ksd", [128, 8], FP32, kind="ExternalInput"),
        'bmap': nc.dram_tensor("bmap", [4, 128], FP32, kind="ExternalInput"),
        'bias': nc.dram_tensor("bias", [128, 8], FP32, kind="ExternalInput"),
        # bias cols: 0-1 alpha1*b_proj (2 chunks), 2-5 b_fc1 (4), 6-7 alpha2*b_fc2
    }
    out = nc.dram_tensor("outT", [C, R], FP32, kind="ExternalOutput")

    with nc.allow_low_precision(reason="f32r intermediate tiles are intended"), \
         tile.TileContext(nc) as tc:
        with tc.tile_pool(name="const", bufs=1) as cpool, \
             tc.tile_pool(name="work", bufs=2) as work, \
             tc.tile_pool(name="psum", bufs=7, space="PSUM") as psum:
            consts = {
                'F': _round_tile(nc, cpool, "F", io['F'], [3, 128]),
                'tbl': _round_tile(nc, cpool, "tbl", io['tbl'], [6, C]),
                'wq0': _round_tile(nc, cpool, "wq0", io['w_q'][0:128, :], [128, C]),
                'wq1': _round_tile(nc, cpool, "wq1", io['w_q'][128:256, :], [128, C]),
                'wp0': _round_tile(nc, cpool, "wp0", io['w_proj'][0:128, :], [128, C]),
                'wp1': _round_tile(nc, cpool, "wp1", io['w_proj'][128:256, :], [128, C]),
                'f10': _round_tile(nc, cpool, "f10", io['w_fc1'][0:128, :], [128, 2 * C]),
                'f11': _round_tile(nc, cpool, "f11", io['w_fc1'][128:256, :], [128, 2 * C]),
                'f20': _round_tile(nc, cpool, "f20", io['w_fc2'][0:128, :], [128, C]),
                'f21': _round_tile(nc, cpool, "f21", io['w_fc2'][128:256, :], [128, C]),
                'f22': _round_tile(nc, cpool, "f22", io['w_fc2'][256:384, :], [128, C]),
                'f23': _round_tile(nc, cpool, "f23", io['w_fc2'][384:512, :], [128, C]),
                'kvd': _round_tile(nc, cpool, "kvd", io['kvd'], [128, 256]),
                'ksd': _round_tile(nc, cpool, "ksd", io['ksd'], [128, 8]),
                'bmap': _round_tile(nc, cpool, "bmap", io['bmap'], [4, 128]),
            }
            bias = cpool.tile([128, 8], FP32)
            nc.sync.dma_start(bias[:], io['bias'][:])
            fc1w = [consts['f10'], consts['f11']]
            fc2w = [consts['f20'], consts['f21'], consts['f22'], consts['f23']]

            for i in range(NT):
                xt0, xt1, x1_0, x1_1 = _emit_x1(nc, tc, io, consts, work, psum, i)
                xts = [xt0, xt1]
                ys = []
                for g in range(2):
                    gs = bass.ts(g, 128)
                    pq = psum.tile([128, T], FP32, tag="ps")
                    nc.tensor.matmul(pq[:], consts['wq0'][:, gs], x1_0[:],
                                     start=True, stop=False)
                    nc.tensor.matmul(pq[:], consts['wq1'][:, gs], x1_1[:],
                                     start=False, stop=True)
                    rq = work.tile([128, T], FP32, tag="rq")
                    nc.scalar.activation(rq[:], pq[:], ACT.Relu, scale=-1.0)
                    eq = work.tile([128, T], FP32, tag="eq")
                    nc.scalar.activation(eq[:], rq[:], ACT.Exp, scale=-1.0)
                    qr = work.tile([128, T], F32R, tag="qr")
                    nc.vector.scalar_tensor_tensor(qr[:], pq[:], 0.0, eq[:],
                                                   ALU.max, ALU.add)
                    # z = 1/(q . ksum + eps), broadcast to head blocks
                    zden_t = psum.tile([128, T], FP32, tag="ps", name="zden")
                    zden = zden_t[0:4, :]
                    nc.tensor.matmul(zden[:], consts['ksd'][:, bass.ts(g, 4)], qr[:],
                                     start=True, stop=True)
                    zr = work.tile([4, T], F32R, tag="zr")
                    ztmp = work.tile([4, T], FP32, tag="ztmp")
                    nc.vector.tensor_scalar_add(ztmp[:], zden[:], EPS)
                    nc.vector.reciprocal(zr[:], ztmp[:])
                    zb = psum.tile([128, T], FP32, tag="ps")
                    nc.tensor.matmul(zb[:], consts['bmap'][:], zr[:],
                                     start=True, stop=True)
                    zbs = work.tile([128, T], FP32, tag="zbs")
                    nc.scalar.activation(zbs[:], zb[:], ACT.Copy)
                    py = psum.tile([128, T], FP32, tag="ps")
                    nc.tensor.matmul(py[:], consts['kvd'][:, gs], qr[:],
                                     start=True, stop=True)
                    y = work.tile([128, T], F32R, tag=f"y{g}")
                    nc.vector.tensor_tensor(y[:], py[:], zbs[:], ALU.mult)
                    ys.append(y)
                x2s = []
                for m in range(2):
                    ms = bass.ts(m, 128)
                    pa = psum.tile([128, T], FP32, tag="ps")
                    nc.tensor.matmul(pa[:], consts['wp0'][:, ms], ys[0][:],
                                     start=True, stop=False)
                    nc.tensor.matmul(pa[:], consts['wp1'][:, ms], ys[1][:],
                                     start=False, stop=True)
                    att = work.tile([128, T], FP32, tag="att")
                    nc.scalar.activation(att[:], pa[:], ACT.Identity,
                                         bias=bias[:, m:m + 1], scale=1.0)
                    x2r = work.tile([128, T], F32R, tag=f"x2r{m}")
                    nc.vector.tensor_tensor(x2r[:], att[:], xts[m][:], ALU.add)
                    x2s.append((x2r, att))
                hs_t = []
                for j in range(4):
                    js = bass.ts(j, 128)
                    phh = psum.tile([128, T], FP32, tag="ps")
                    nc.tensor.matmul(phh[:], fc1w[0][:, js], x2s[0][0][:],
                                     start=True, stop=False)
                    nc.tensor.matmul(phh[:], fc1w[1][:, js], x2s[1][0][:],
                                     start=False, stop=True)
                    hj = work.tile([128, T], F32R, tag=f"hj{j}")
                    nc.scalar.activation(hj[:], phh[:], ACT.Gelu,
                                         bias=bias[:, 2 + j:3 + j], scale=1.0)
                    hs_t.append(hj)
                for m in range(2):
                    ms = bass.ts(m, 128)
                    po = psum.tile([128, T], FP32, tag="ps")
                    for j in range(4):
                        nc.tensor.matmul(po[:], fc2w[j][:, ms], hs_t[j][:],
                                         start=(j == 0), stop=(j == 3))
                    mo = work.tile([128, T], FP32, tag="mo")
                    nc.scalar.activation(mo[:], po[:], ACT.Identity,
                                         bias=bias[:, 6 + m:7 + m], scale=1.0)
                    t = work.tile([128, T], FP32, tag="ot1")
                    nc.vector.tensor_tensor(t[:], mo[:], x2s[m][1][:], ALU.add)
                    ot = work.tile([128, T], FP32, tag="ot2")
                    nc.vector.tensor_tensor(ot[:], t[:], xts[m][:], ALU.add)
                    nc.sync.dma_start(out[bass.ts(m, 128), bass.ts(i, T)], ot[:])
    nc.finalize()
    return nc


_NC_CACHE = {}
EXEC_NS = []


def _get_nc(name):
    if name not in _NC_CACHE:
        _NC_CACHE[name] = build_phase1() if name == 'p1' else build_phase2()
    return _NC_CACHE[name]


# ----------------------------------------------------------------- host ---
def _sine2_np(u, v, nf, scale):
    dim_t = 10000.0 ** (2.0 * np.floor(np.arange(nf) / 2.0) / nf)
    pu = u[..., None] / dim_t * scale
    pv = v[..., None] / dim_t * scale
    def emb(p):
        return np.stack([np.sin(p[..., 0::2]), np.cos(p[..., 1::2])], axis=-1
                        ).reshape(*p.shape[:-1], -1)
    return np.concatenate([emb(pv), emb(pu)], axis=-1)


def _sine1_np(s, nf, scale):
    dim_t = 10000.0 ** (2.0 * np.floor(np.arange(nf) / 2.0) / nf)
    p = s[..., None] / dim_t * scale
    return np.stack([np.sin(p[..., 0::2]), np.cos(p[..., 1::2])], axis=-1
                    ).reshape(*p.shape[:-1], -1)


def _host_prep(x, epipole, tok_table):
    """Per-core xT/rel/sel shards + per-batch const tables."""
    xr = np.asarray(x, np.float32).reshape(B, L, C)
    ep = np.asarray(epipole, np.float64)
    tt = np.asarray(tok_table, np.float32)

    g = np.arange(L)
    v_idx = g // HW
    pos = g % HW
    n_idx = np.maximum(v_idx - 1, 0)
    p = np.maximum(pos - 1, 0)
    py, px = (p // Ww).astype(np.float64), (p % Ww).astype(np.float64)
    is_pix = (v_idx > 0) & (pos > 0)

    shards = []
    tbls, Fs = [], None
    # rel_emb frequencies (ch 128:256): w_i = 32pi / 10000^(2i/64), i<32
    nf = C // 4
    dim_t = 10000.0 ** (2.0 * np.floor(np.arange(nf) / 2.0) / nf)
    w = (32 * math.pi) / dim_t  # length 64, paired
    F = np.zeros((3, 128), np.float64)
    j = np.arange(64)
    F[0, :64] = w
    F[1, 64:] = w
    F[2, :] = np.where(np.tile(j, 2) % 2 == 1, math.pi / 2, 0.0)
    Fs = F.astype(np.float32)

    for b in range(B):
        eu = ep[b, :, 0][n_idx]
        ev = ep[b, :, 1][n_idx]
        ru_raw = px - eu
        rv_raw = py - ev
        nrm = np.sqrt(ru_raw ** 2 + rv_raw ** 2)
        ru = np.where(is_pix, ru_raw / (nrm + 1e-6), 0.0)
        rv = np.where(is_pix, rv_raw / (nrm + 1e-6), 0.0)
        mask = is_pix.astype(np.float64)

        sel_row = np.where(v_idx == 0, 0, np.where(pos == 0, 1, 2 + n_idx))
        sel = np.zeros((6, L), np.float32)
        sel[sel_row, g] = 1.0

        tbl = np.zeros((6, C), np.float32)
        tbl[0] = tt[0]
        tbl[1] = tt[1]
        en = np.sqrt(ep[b, :, 0] ** 2 + ep[b, :, 1] ** 2)
        enorm = np.maximum(en, 1e-12)
        dir_e = _sine2_np(ep[b, :, 0] / enorm, ep[b, :, 1] / enorm, C // 8, 2 * math.pi)
        dis = np.clip(en / 512.0, 0.0, 1.0)
        dis_e = _sine1_np(dis, C // 4, 2 * math.pi)
        tbl[2:6, 0:64] = dir_e
        tbl[2:6, 64:128] = dis_e
        tbls.append(tbl)

        xb = xr[b].T  # [C, L]
        for s in range(4):
            lo, hi = s * R, min((s + 1) * R, L)
            n = hi - lo
            xT = np.zeros((C, R), np.float32); xT[:, :n] = xb[:, lo:hi]
            rel = np.zeros((3, R), np.float32)
            rel[0, :n] = rv[lo:hi]; rel[1, :n] = ru[lo:hi]; rel[2, :n] = mask[lo:hi]
            selp = np.zeros((6, R), np.float32); selp[:, :n] = sel[:, lo:hi]
            shards.append({'xT': xT, 'rel': rel, 'sel': selp})
    return shards, tbls, Fs


def kernel(x, epipole, w_qkv, w_proj, b_proj, w_fc1, b_fc1, w_fc2, b_fc2,
           tok_table, alpha1, alpha2, height, width):
    assert int(height) == Hh and int(width) == Ww
    x = np.asarray(x, np.float32)
    w_qkv = np.asarray(w_qkv, np.float32)
    shards, tbls, F = _host_prep(x, epipole, tok_table)

    w_kv = np.ascontiguousarray(w_qkv[:, C:3 * C])
    in1 = []
    for ci in range(8):
        b = ci // 4
        m = dict(shards[ci])
        m['F'] = F
        m['tbl'] = tbls[b]
        m['w_kv'] = w_kv
        in1.append(m)
    nc1 = _get_nc('p1')
    _tr = bool(os.environ.get('KTRACE'))
    res1 = run_bass_kernel_spmd(nc1, in1, core_ids=list(range(8)), trace=_tr)
    EXEC_NS.clear()
    if res1.exec_time_ns:
        EXEC_NS.append(res1.exec_time_ns)

    n_pad = 4 * R - L
    kv_b, ks_b = [], []
    for b in range(2):
        kvA = sum(res1.results[4 * b + s]['kvA'].astype(np.float64) for s in range(4))
        kvB = sum(res1.results[4 * b + s]['kvB'].astype(np.float64) for s in range(4))
        ks = sum(res1.results[4 * b + s]['ks'].astype(np.float64) for s in range(4))
        ks = ks - n_pad  # remove pad-token contribution (k_pad = exactly 1)
        kv = np.zeros((32, C))
        for h in range(8):
            blk = (kvA if h < 4 else kvB)[32 * (h % 4):32 * (h % 4 + 1),
                                          32 * h:32 * (h + 1)]
            kv[:, 32 * h:32 * (h + 1)] = blk
        kv_b.append(kv)
        ks_b.append(ks[0])

    a1 = np.float32(alpha1); a2 = np.float32(alpha2)
    bias = np.zeros((128, 8), np.float32)
    bias[:, 0] = a1 * np.asarray(b_proj)[0:128]
    bias[:, 1] = a1 * np.asarray(b_proj)[128:256]
    for j in range(4):
        bias[:, 2 + j] = np.asarray(b_fc1)[128 * j:128 * (j + 1)]
    bias[:, 6] = a2 * np.asarray(b_fc2)[0:128]
    bias[:, 7] = a2 * np.asarray(b_fc2)[128:256]

    in2 = []
    for ci in range(8):
        b = ci // 4
        # kv[h][m,d] at kv_b rows m(0:32), cols 32h+d ; lhsT needs [32h'+d, 32h'+m]
        kvd = np.zeros((128, 256), np.float32)
        ksd = np.zeros((128, 8), np.float32)
        for g in range(2):
            for hp in range(4):
                h = 4 * g + hp
                blk = kv_b[b][:, 32 * h:32 * (h + 1)]  # [m, d]
                kvd[32 * hp:32 * (hp + 1), 128 * g + 32 * hp:128 * g + 32 * (hp + 1)] = \
                    blk.T.astype(np.float32)
                ksd[32 * hp:32 * (hp + 1), 4 * g + hp] = \
                    ks_b[b][32 * h:32 * (h + 1)].astype(np.float32)
        bmap = np.zeros((4, 128), np.float32)
        for hp in range(4):
            bmap[hp, 32 * hp:32 * (hp + 1)] = 1.0
        m = dict(shards[ci])
        m.update({'F': F, 'tbl': tbls[b],
                  'w_q': np.ascontiguousarray(w_qkv[:, 0:C]),
                  'w_proj': np.asarray(w_proj, np.float32) * a1,
                  'w_fc1': np.asarray(w_fc1, np.float32),
                  'w_fc2': np.asarray(w_fc2, np.float32) * a2,
                  'kvd': kvd, 'ksd': ksd, 'bmap': bmap, 'bias': bias})
        in2.append(m)
    nc2 = _get_nc('p2')
    res2 = run_bass_kernel_spmd(nc2, in2, core_ids=list(range(8)), trace=_tr)
    if res2.exec_time_ns:
        EXEC_NS.append(res2.exec_time_ns)

    out = np.empty((B, L, C), np.float32)
    for ci in range(8):
        b, s = ci // 4, ci % 4
        lo, hi = s * R, min((s + 1) * R, L)
        out[b, lo:hi] = res2.results[ci]['outT'][:, :hi - lo].T
    return out.reshape(B * V, HW, C)


# revision 12
# speedup vs baseline: 1.0460x; 1.0460x over previous
"""CrossBlock (sine pos-emb + linear elu+1 attention + MLP) on 8 trn2 cores.

Sharding: tokens of each batch element (V*HW = 24005) split over 4 cores
(cores 0-3 = batch 0, cores 4-7 = batch 1), padded to R = 6144 per core.
Two SPMD launches: phase 1 computes per-shard partial kv = sum_l k v^T and
ksum = sum_l k (33 KB); the host reduces those across each batch's 4 cores;
phase 2 computes q, y = (q @ kv) * z, proj, and the MLP.  Everything runs in
[channel, token] (transposed) layout so no on-chip transposes are needed:
the host supplies x^T shards and transposes the output back.
"""
import sys, os, json, math
sys.path.insert(0, '/opt/trn_rl_repo')
import numpy as np

import concourse.bass as bass
import concourse.mybir as mybir
import concourse.tile as tile
from concourse.bass_utils import run_bass_kernel_spmd

FP32 = mybir.dt.float32
F32R = mybir.dt.float32r
BF16 = mybir.dt.bfloat16
ACT = mybir.ActivationFunctionType
ALU = mybir.AluOpType

B, V, Hh, Ww, C, NH = 2, 5, 60, 80, 256, 8
HW = Hh * Ww + 1
L = V * HW          # 24005 tokens per batch element
R = 6144            # tokens per core (padded); 4 cores per batch
T = 512             # token tile
NT = R // T
NCHUNK = T // 128   # 128-token chunks per tile
EPS = 1e-6
MAGIC = 12582912.0  # 1.5 * 2^23 fp32 round-to-nearest trick
TWO_PI = 2.0 * math.pi

# ---------------------------------------------------------------- bir fix --
def _fix_inst_list(lst, counter):
    out = []
    for ins in lst:
        if not (isinstance(ins, dict) and 'opcode' in ins and 'sync_info' in ins):
            out.append(ins); continue
        si = ins.get('sync_info') or {}
        waits = si.get('on_wait') or []
        ups = si.get('on_update') or []
        if len(waits) > 1:
            for w in waits[:-1]:
                counter[0] += 1
                out.append({"debug": ins.get("debug", 0), "engine": ins["engine"],
                            "ins": [], "outs": [], "name": f"I-wfix{counter[0]}",
                            "opcode": "EventSemaphore",
                            "sync_info": {"on_update": [], "on_wait": [w]}})
            si['on_wait'] = [waits[-1]]
        out.append(ins)
        if len(ups) > 1:
            si['on_update'] = [ups[0]]
            for u in ups[1:]:
                counter[0] += 1
                out.append({"debug": ins.get("debug", 0), "engine": ins["engine"],
                            "ins": [], "outs": [], "name": f"I-ufix{counter[0]}",
                            "opcode": "EventSemaphore",
                            "sync_info": {"on_update": [u], "on_wait": []}})
    return out


def _walk(o, counter):
    if isinstance(o, dict):
        for k, v in o.items():
            if isinstance(v, list) and v and isinstance(v[0], dict) and 'opcode' in v[0]:
                o[k] = _fix_inst_list(v, counter)
                for ins in o[k]:
                    _walk(ins, counter)
            else:
                _walk(v, counter)
    elif isinstance(o, list):
        for v in o:
            _walk(v, counter)


def _install_bir_fix():
    if getattr(bass.Bass, '_birfix_installed', False):
        return
    orig = bass.Bass.to_json_bytes

    def patched(self):
        m = json.loads(orig(self))
        _walk(m, [0])
        return json.dumps(m).encode()

    bass.Bass.to_json_bytes = patched
    bass.Bass._birfix_installed = True


_install_bir_fix()

# ------------------------------------------------------------- emit shared --
def _round_tile(nc, pool, name, src_dram, shape, dt=F32R):
    """DMA a small dram tensor to SBUF and produce a rounded (f32r) copy."""
    stg_full = pool.tile([128, 512], FP32, tag="stg", name=f"stg_{name}")
    stg = stg_full[:shape[0], :shape[1]]
    nc.sync.dma_start(stg[:], src_dram[:])
    r = pool.tile(shape, dt, tag=f"r_{name}")
    nc.vector.tensor_copy(r[:], stg[:])
    return r


def _emit_x1(nc, tc, io, consts, work, psum, i):
    """Emit x1^T = x^T + tok_emb^T for token tile i; returns (x1_0, x1_1) f32r."""
    sl = bass.ts(i, T)
    xt0 = work.tile([128, T], FP32, tag="xt0")
    nc.sync.dma_start(xt0[:], io['xT'][0:128, sl])
    xt1 = work.tile([128, T], FP32, tag="xt1")
    nc.sync.dma_start(xt1[:], io['xT'][128:256, sl])
    rel = work.tile([3, T], FP32, tag="rel")
    nc.sync.dma_start(rel[:], io['rel'][:, sl])
    sel = work.tile([6, T], FP32, tag="sel")
    nc.sync.dma_start(sel[:], io['sel'][:, sl])
    rel_r = work.tile([3, T], F32R, tag="rel_r")
    nc.vector.tensor_copy(rel_r[:], rel[:])
    sel_r = work.tile([6, T], F32R, tag="sel_r")
    nc.vector.tensor_copy(sel_r[:], sel[:])

    # phase matrix P = F.T @ [rel_v; rel_u; mask]  -> [128, T] (channels 128:256)
    ph = psum.tile([128, T], FP32, tag="ps")
    nc.tensor.matmul(ph[:], consts['F'][:], rel_r[:], start=True, stop=True)
    # range-reduce: x' = P - 2pi*round(P/2pi)   (ACT for the two scalar steps)
    t1 = work.tile([128, T], FP32, tag="sr1")
    nc.vector.tensor_scalar(t1[:], ph[:], 1.0 / TWO_PI, MAGIC, ALU.mult, ALU.add)
    t2 = work.tile([128, T], FP32, tag="sr2")
    nc.vector.tensor_scalar(t2[:], t1[:], MAGIC, -TWO_PI, ALU.subtract, ALU.mult)
    t3 = work.tile([128, T], FP32, tag="sr3")
    nc.vector.tensor_tensor(t3[:], t2[:], ph[:], ALU.add)
    sinp = work.tile([128, T], FP32, tag="sinp")
    nc.scalar.activation(sinp[:], t3[:], ACT.Sin)

    # const part (two 128-channel chunks) from table
    c0 = psum.tile([128, T], FP32, tag="ps")
    nc.tensor.matmul(c0[:], consts['tbl'][:, 0:128], sel_r[:], start=True, stop=True)
    c1 = psum.tile([128, T], FP32, tag="ps")
    nc.tensor.matmul(c1[:], consts['tbl'][:, 128:256], sel_r[:], start=True, stop=True)

    x1_0 = work.tile([128, T], F32R, tag="x1_0")
    nc.vector.tensor_tensor(x1_0[:], xt0[:], c0[:], ALU.add)
    tmp = work.tile([128, T], FP32, tag="x1tmp")
    nc.vector.tensor_tensor(tmp[:], xt1[:], c1[:], ALU.add)
    x1_1 = work.tile([128, T], F32R, tag="x1_1")
    nc.vector.tensor_tensor(x1_1[:], tmp[:], sinp[:], ALU.add)
    return xt0, xt1, x1_0, x1_1


# --------------------------------------------------------------- phase 1 --
def build_phase1():
    nc = bass.Bass()
    io = {
        'xT': nc.dram_tensor("xT", [C, R], FP32, kind="ExternalInput"),
        'rel': nc.dram_tensor("rel", [3, R], FP32, kind="ExternalInput"),
        'sel': nc.dram_tensor("sel", [6, R], FP32, kind="ExternalInput"),
        'F': nc.dram_tensor("F", [3, 128], FP32, kind="ExternalInput"),
        'tbl': nc.dram_tensor("tbl", [6, C], FP32, kind="ExternalInput"),
        'w_kv': nc.dram_tensor("w_kv", [C, 2 * C], FP32, kind="ExternalInput"),
    }
    kv_outA = nc.dram_tensor("kvA", [128, C], FP32, kind="ExternalOutput")
    kv_outB = nc.dram_tensor("kvB", [128, C], FP32, kind="ExternalOutput")
    ks_out = nc.dram_tensor("ks", [1, C], FP32, kind="ExternalOutput")

    with nc.allow_low_precision(reason="bf16 kv accumulation is intended"), \
         tile.TileContext(nc) as tc:
        with tc.tile_pool(name="const", bufs=1) as cpool, \
             tc.tile_pool(name="work", bufs=3) as work, \
             tc.tile_pool(name="acc", bufs=1, space="PSUM") as accp, \
             tc.tile_pool(name="psum", bufs=4, space="PSUM") as psum:
            consts = {
                'F': _round_tile(nc, cpool, "F", io['F'], [3, 128]),
                'tbl': _round_tile(nc, cpool, "tbl", io['tbl'], [6, C]),
                'wkv0': _round_tile(nc, cpool, "wkv0", io['w_kv'][0:128, :], [128, 2 * C]),
                'wkv1': _round_tile(nc, cpool, "wkv1", io['w_kv'][128:256, :], [128, 2 * C]),
            }
            ones_s = cpool.tile([128, 1], FP32)
            nc.vector.memset(ones_s[:], 1.0)
            ones = cpool.tile([128, 1], F32R)
            nc.vector.tensor_copy(ones[:], ones_s[:])
            pkvA = accp.tile([128, C], FP32)
            pkvB = accp.tile([128, C], FP32)
            pks = accp.tile([1, C], FP32)

            nmm = NT * NCHUNK
            mm = 0
            for i in range(NT):
                _, _, x1_0, x1_1 = _emit_x1(nc, tc, io, consts, work, psum, i)
                for cch in range(NCHUNK):
                    csl = bass.ts(cch, 128)
                    # natural-layout k|v for these 128 tokens: [tok, 512]
                    kvn = psum.tile([128, 2 * C], FP32, tag="ps")
                    nc.tensor.matmul(kvn[:], x1_0[:, csl], consts['wkv0'][:],
                                     start=True, stop=False)
                    nc.tensor.matmul(kvn[:], x1_1[:, csl], consts['wkv1'][:],
                                     start=False, stop=True)
                    # k = elu(.)+1 in bf16; v plain bf16
                    r1 = work.tile([128, C], FP32, tag="r1")
                    nc.scalar.activation(r1[:], kvn[:, 0:C], ACT.Relu, scale=-1.0)
                    e1 = work.tile([128, C], FP32, tag="e1")
                    nc.scalar.activation(e1[:], r1[:], ACT.Exp, scale=-1.0)
                    k_bf = work.tile([128, C], F32R, tag="k_bf")
                    nc.vector.scalar_tensor_tensor(k_bf[:], kvn[:, 0:C], 0.0, e1[:],
                                                   ALU.max, ALU.add)
                    v_bf = work.tile([128, C], F32R, tag="v_bf")
                    nc.vector.tensor_copy(v_bf[:], kvn[:, C:2 * C])
                    first, last = mm == 0, mm == nmm - 1
                    nc.tensor.matmul(pkvA[:], v_bf[:, 0:128], k_bf[:],
                                     start=first, stop=last)
                    nc.tensor.matmul(pkvB[:], v_bf[:, 128:256], k_bf[:],
                                     start=first, stop=last)
                    nc.tensor.matmul(pks[:], ones[:], k_bf[:], start=first, stop=last)
                    mm += 1
            okvA = cpool.tile([128, C], FP32, tag="okvA")
            nc.vector.tensor_copy(okvA[:], pkvA[:])
            nc.sync.dma_start(kv_outA[:], okvA[:])
            okvB = cpool.tile([128, C], FP32, tag="okvB")
            nc.vector.tensor_copy(okvB[:], pkvB[:])
            nc.sync.dma_start(kv_outB[:], okvB[:])
            oks = cpool.tile([1, C], FP32, tag="oks")
            nc.vector.tensor_copy(oks[:], pks[:])
            nc.sync.dma_start(ks_out[:], oks[:])
    nc.finalize()
    return nc


# --------------------------------------------------------------- phase 2 --
def build_phase2():
    nc = bass.Bass()
    io = {
        'xT': nc.dram_tensor("xT", [C, R], FP32, kind="ExternalInput"),
        'rel': nc.dram_tensor("rel", [3, R], FP32, kind="ExternalInput"),
        'sel': nc.dram_tensor("sel", [6, R], FP32, kind="ExternalInput"),
        'F': nc.dram_tensor("F", [3, 128], FP32, kind="ExternalInput"),
        'tbl': nc.dram_tensor("tbl", [6, C], FP32, kind="ExternalInput"),
        'w_q': nc.dram_tensor("w_q", [C, C], FP32, kind="ExternalInput"),
        'w_proj': nc.dram_tensor("w_proj", [C, C], FP32, kind="ExternalInput"),
        'w_fc1': nc.dram_tensor("w_fc1", [C, 2 * C], FP32, kind="ExternalInput"),
        'w_fc2': nc.dram_tensor("w_fc2", [2 * C, C], FP32, kind="ExternalInput"),
        'kvd': nc.dram_tensor("kvd", [128, 2 * 128], FP32, kind="ExternalInput"),
        'ksd': nc.dram_tensor("ksd", [128, 8], FP32, kind="ExternalInput"),
        'bmap': nc.dram_tensor("bmap", [4, 128], FP32, kind="ExternalInput"),
        'bias': nc.dram_tensor("bias", [128, 8], FP32, kind="ExternalInput"),
        # bias cols: 0-1 alpha1*b_proj (2 chunks), 2-5 b_fc1 (4), 6-7 alpha2*b_fc2
    }
    out = nc.dram_tensor("outT", [C, R], FP32, kind="ExternalOutput")

    with nc.allow_low_precision(reason="f32r intermediate tiles are intended"), \
         tile.TileContext(nc) as tc:
        with tc.tile_pool(name="const", bufs=1) as cpool, \
             tc.tile_pool(name="work", bufs=3) as work, \
             tc.tile_pool(name="psum", bufs=7, space="PSUM") as psum:
            consts = {
                'F': _round_tile(nc, cpool, "F", io['F'], [3, 128]),
                'tbl': _round_tile(nc, cpool, "tbl", io['tbl'], [6, C]),
                'wq0': _round_tile(nc, cpool, "wq0", io['w_q'][0:128, :], [128, C]),
                'wq1': _round_tile(nc, cpool, "wq1", io['w_q'][128:256, :], [128, C]),
                'wp0': _round_tile(nc, cpool, "wp0", io['w_proj'][0:128, :], [128, C]),
                'wp1': _round_tile(nc, cpool, "wp1", io['w_proj'][128:256, :], [128, C]),
                'f10': _round_tile(nc, cpool, "f10", io['w_fc1'][0:128, :], [128, 2 * C]),
                'f11': _round_tile(nc, cpool, "f11", io['w_fc1'][128:256, :], [128, 2 * C]),
                'f20': _round_tile(nc, cpool, "f20", io['w_fc2'][0:128, :], [128, C]),
                'f21': _round_tile(nc, cpool, "f21", io['w_fc2'][128:256, :], [128, C]),
                'f22': _round_tile(nc, cpool, "f22", io['w_fc2'][256:384, :], [128, C]),
                'f23': _round_tile(nc, cpool, "f23", io['w_fc2'][384:512, :], [128, C]),
                'kvd': _round_tile(nc, cpool, "kvd", io['kvd'], [128, 256]),
                'ksd': _round_tile(nc, cpool, "ksd", io['ksd'], [128, 8]),
                'bmap': _round_tile(nc, cpool, "bmap", io['bmap'], [4, 128]),
            }
            bias = cpool.tile([128, 8], FP32)
            nc.sync.dma_start(bias[:], io['bias'][:])
            fc1w = [consts['f10'], consts['f11']]
            fc2w = [consts['f20'], consts['f21'], consts['f22'], consts['f23']]

            for i in range(NT):
                xt0, xt1, x1_0, x1_1 = _emit_x1(nc, tc, io, consts, work, psum, i)
                xts = [xt0, xt1]
                ys = []
                for g in range(2):
                    gs = bass.ts(g, 128)
                    pq = psum.tile([128, T], FP32, tag="ps")
                    nc.tensor.matmul(pq[:], consts['wq0'][:, gs], x1_0[:],
                                     start=True, stop=False)
                    nc.tensor.matmul(pq[:], consts['wq1'][:, gs], x1_1[:],
                                     start=False, stop=True)
                    rq = work.tile([128, T], FP32, tag="rq")
                    nc.scalar.activation(rq[:], pq[:], ACT.Relu, scale=-1.0)
                    eq = work.tile([128, T], FP32, tag="eq")
                    nc.scalar.activation(eq[:], rq[:], ACT.Exp, scale=-1.0)
                    qr = work.tile([128, T], F32R, tag="qr")
                    nc.vector.scalar_tensor_tensor(qr[:], pq[:], 0.0, eq[:],
                                                   ALU.max, ALU.add)
                    # z = 1/(q . ksum + eps), broadcast to head blocks
                    zden_t = psum.tile([128, T], FP32, tag="ps", name="zden")
                    zden = zden_t[0:4, :]
                    nc.tensor.matmul(zden[:], consts['ksd'][:, bass.ts(g, 4)], qr[:],
                                     start=True, stop=True)
                    zr = work.tile([4, T], F32R, tag="zr")
                    ztmp = work.tile([4, T], FP32, tag="ztmp")
                    nc.vector.tensor_scalar_add(ztmp[:], zden[:], EPS)
                    nc.vector.reciprocal(zr[:], ztmp[:])
                    zb = psum.tile([128, T], FP32, tag="ps")
                    nc.tensor.matmul(zb[:], consts['bmap'][:], zr[:],
                                     start=True, stop=True)
                    zbs = work.tile([128, T], FP32, tag="zbs")
                    nc.scalar.activation(zbs[:], zb[:], ACT.Copy)
                    py = psum.tile([128, T], FP32, tag="ps")
                    nc.tensor.matmul(py[:], consts['kvd'][:, gs], qr[:],
                                     start=True, stop=True)
                    y = work.tile([128, T], F32R, tag=f"y{g}")
                    nc.vector.tensor_tensor(y[:], py[:], zbs[:], ALU.mult)
                    ys.append(y)
                x2s = []
                for m in range(2):
                    ms = bass.ts(m, 128)
                    pa = psum.tile([128, T], FP32, tag="ps")
                    nc.tensor.matmul(pa[:], consts['wp0'][:, ms], ys[0][:],
                                     start=True, stop=False)
                    nc.tensor.matmul(pa[:], consts['wp1'][:, ms], ys[1][:],
                                     start=False, stop=True)
                    att = work.tile([128, T], FP32, tag="att")
                    nc.scalar.activation(att[:], pa[:], ACT.Identity,
                                         bias=bias[:, m:m + 1], scale=1.0)
                    x2r = work.tile([128, T], F32R, tag=f"x2r{m}")
                    nc.vector.tensor_tensor(x2r[:], att[:], xts[m][:], ALU.add)
                    x2s.append((x2r, att))
                hs_t = []
                for j in range(4):
                    js = bass.ts(j, 128)
                    phh = psum.tile([128, T], FP32, tag="ps")
                    nc.tensor.matmul(phh[:], fc1w[0][:, js], x2s[0][0][:],
                                     start=True, stop=False)
                    nc.tensor.matmul(phh[:], fc1w[1][:, js], x2s[1][0][:],
                                     start=False, stop=True)
                    hj = work.tile([128, T], F32R, tag=f"hj{j}")
                    nc.scalar.activation(hj[:], phh[:], ACT.Gelu,
                                         bias=bias[:, 2 + j:3 + j], scale=1.0)
                    hs_t.append(hj)
                for m in range(2):
                    ms = bass.ts(m, 128)
                    po = psum.tile([128, T], FP32, tag="ps")
                    for j in range(4):
                        nc.tensor.matmul(po[:], fc2w[j][:, ms], hs_t[j][:],
                                         start=(j == 0), stop=(j == 3))
                    mo = work.tile([128, T], FP32, tag="mo")
                    nc.scalar.activation(mo[:], po[:], ACT.Identity,
                                         bias=bias[:, 6 + m:7 + m], scale=1.0)
                    t = work.tile([128, T], FP32, tag="ot1")
                    nc.vector.tensor_tensor(t[:], mo[:], x2s[m][1][:], ALU.add)
                    ot = work.tile([128, T], FP32, tag="ot2")
                    nc.vector.tensor_tensor(ot[:], t[:], xts[m][:], ALU.add)
                    nc.sync.dma_start(out[bass.ts(m, 128), bass.ts(i, T)], ot[:])
    nc.finalize()
    return nc


_NC_CACHE = {}
EXEC_NS = []


def _get_nc(name):
    if name not in _NC_CACHE:
        _NC_CACHE[name] = build_phase1() if name == 'p1' else build_phase2()
    return _NC_CACHE[name]


# ----------------------------------------------------------------- host ---
def _sine2_np(u, v, nf, scale):
    dim_t = 10000.0 ** (2.0 * np.floor(np.arange(nf) / 2.0) / nf)
    pu = u[..., None] / dim_t * scale
    pv = v[..., None] / dim_t * scale
    def emb(p):
        return np.stack([np.sin(p[..., 0::2]), np.cos(p[..., 1::2])], axis=-1
                        ).reshape(*p.shape[:-1], -1)
    return np.concatenate([emb(pv), emb(pu)], axis=-1)


def _sine1_np(s, nf, scale):
    dim_t = 10000.0 ** (2.0 * np.floor(np.arange(nf) / 2.0) / nf)
    p = s[..., None] / dim_t * scale
    return np.stack([np.sin(p[..., 0::2]), np.cos(p[..., 1::2])], axis=-1
                    ).reshape(*p.shape[:-1], -1)


def _host_prep(x, epipole, tok_table):
    """Per-core xT/rel/sel shards + per-batch const tables."""
    xr = np.asarray(x, np.float32).reshape(B, L, C)
    ep = np.asarray(epipole, np.float64)
    tt = np.asarray(tok_table, np.float32)

    g = np.arange(L)
    v_idx = g // HW
    pos = g % HW
    n_idx = np.maximum(v_idx - 1, 0)
    p = np.maximum(pos - 1, 0)
    py, px = (p // Ww).astype(np.float64), (p % Ww).astype(np.float64)
    is_pix = (v_idx > 0) & (pos > 0)

    shards = []
    tbls, Fs = [], None
    # rel_emb frequencies (ch 128:256): w_i = 32pi / 10000^(2i/64), i<32
    nf = C // 4
    dim_t = 10000.0 ** (2.0 * np.floor(np.arange(nf) / 2.0) / nf)
    w = (32 * math.pi) / dim_t  # length 64, paired
    F = np.zeros((3, 128), np.float64)
    j = np.arange(64)
    F[0, :64] = w
    F[1, 64:] = w
    F[2, :] = np.where(np.tile(j, 2) % 2 == 1, math.pi / 2, 0.0)
    Fs = F.astype(np.float32)

    for b in range(B):
        eu = ep[b, :, 0][n_idx]
        ev = ep[b, :, 1][n_idx]
        ru_raw = px - eu
        rv_raw = py - ev
        nrm = np.sqrt(ru_raw ** 2 + rv_raw ** 2)
        ru = np.where(is_pix, ru_raw / (nrm + 1e-6), 0.0)
        rv = np.where(is_pix, rv_raw / (nrm + 1e-6), 0.0)
        mask = is_pix.astype(np.float64)

        sel_row = np.where(v_idx == 0, 0, np.where(pos == 0, 1, 2 + n_idx))
        sel = np.zeros((6, L), np.float32)
        sel[sel_row, g] = 1.0

        tbl = np.zeros((6, C), np.float32)
        tbl[0] = tt[0]
        tbl[1] = tt[1]
        en = np.sqrt(ep[b, :, 0] ** 2 + ep[b, :, 1] ** 2)
        enorm = np.maximum(en, 1e-12)
        dir_e = _sine2_np(ep[b, :, 0] / enorm, ep[b, :, 1] / enorm, C // 8, 2 * math.pi)
        dis = np.clip(en / 512.0, 0.0, 1.0)
        dis_e = _sine1_np(dis, C // 4, 2 * math.pi)
        tbl[2:6, 0:64] = dir_e
        tbl[2:6, 64:128] = dis_e
        tbls.append(tbl)

        xb = xr[b].T  # [C, L]
        for s in range(4):
            lo, hi = s * R, min((s + 1) * R, L)
            n = hi - lo
            xT = np.zeros((C, R), np.float32); xT[:, :n] = xb[:, lo:hi]
            rel = np.zeros((3, R), np.float32)
            rel[0, :n] = rv[lo:hi]; rel[1, :n] = ru[lo:hi]; rel[2, :n] = mask[lo:hi]
            selp = np.zeros((6, R), np.float32); selp[:, :n] = sel[:, lo:hi]
            shards.append({'xT': xT, 'rel': rel, 'sel': selp})
    return shards, tbls, Fs


def kernel(x, epipole, w_qkv, w_proj, b_proj, w_fc1, b_fc1, w_fc2, b_fc2,
           tok_table, alpha1, alpha2, height, width):
    assert int(height) == Hh and int(width) == Ww
    x = np.asarray(x, np.float32)
    w_qkv = np.asarray(w_qkv, np.float32)
    shards, tbls, F = _host_prep(x, epipole, tok_table)

    w_kv = np.ascontiguousarray(w_qkv[:, C:3 * C])
    in1 = []
    for ci in range(8):
        b = ci // 4
        m = dict(shards[ci])
        m['F'] = F
        m['tbl'] = tbls[b]
        m['w_kv'] = w_kv
        in1.append(m)
    nc1 = _get_nc('p1')
    _tr = bool(os.environ.get('KTRACE'))
    res1 = run_bass_kernel_spmd(nc1, in1, core_ids=list(range(8)), trace=_tr)
    EXEC_NS.clear()
    if res1.exec_time_ns:
        EXEC_NS.append(res1.exec_time_ns)

    n_pad = 4 * R - L
    kv_b, ks_b = [], []
    for b in range(2):
        kvA = sum(res1.results[4 * b + s]['kvA'].astype(np.float64) for s in range(4))
        kvB = sum(res1.results[4 * b + s]['kvB'].astype(np.float64) for s in range(4))
        ks = sum(res1.results[4 * b + s]['ks'].astype(np.float64) for s in range(4))
        ks = ks - n_pad  # remove pad-token contribution (k_pad = exactly 1)
        kv = np.zeros((32, C))
        for h in range(8):
            blk = (kvA if h < 4 else kvB)[32 * (h % 4):32 * (h % 4 + 1),
                                          32 * h:32 * (h + 1)]
            kv[:, 32 * h:32 * (h + 1)] = blk
        kv_b.append(kv)
        ks_b.append(ks[0])

    a1 = np.float32(alpha1); a2 = np.float32(alpha2)
    bias = np.zeros((128, 8), np.float32)
    bias[:, 0] = a1 * np.asarray(b_proj)[0:128]
    bias[:, 1] = a1 * np.asarray(b_proj)[128:256]
    for j in range(4):
        bias[:, 2 + j] = np.asarray(b_fc1)[128 * j:128 * (j + 1)]
    bias[:, 6] = a2 * np.asarray(b_fc2)[0:128]
    bias[:, 7] = a2 * np.asarray(b_fc2)[128:256]

    in2 = []
    for ci in range(8):
        b = ci // 4
        # kv[h][m,d] at kv_b rows m(0:32), cols 32h+d ; lhsT needs [32h'+d, 32h'+m]
        kvd = np.zeros((128, 256), np.float32)
        ksd = np.zeros((128, 8), np.float32)
        for g in range(2):
            for hp in range(4):
                h = 4 * g + hp
                blk = kv_b[b][:, 32 * h:32 * (h + 1)]  # [m, d]
                kvd[32 * hp:32 * (hp + 1), 128 * g + 32 * hp:128 * g + 32 * (hp + 1)] = \
                    blk.T.astype(np.float32)
                ksd[32 * hp:32 * (hp + 1), 4 * g + hp] = \
                    ks_b[b][32 * h:32 * (h + 1)].astype(np.float32)
        bmap = np.zeros((4, 128), np.float32)
        for hp in range(4):
            bmap[hp, 32 * hp:32 * (hp + 1)] = 1.0
        m = dict(shards[ci])
        m.update({'F': F, 'tbl': tbls[b],
                  'w_q': np.ascontiguousarray(w_qkv[:, 0:C]),
                  'w_proj': np.asarray(w_proj, np.float32) * a1,
                  'w_fc1': np.asarray(w_fc1, np.float32),
                  'w_fc2': np.asarray(w_fc2, np.float32) * a2,
                  'kvd': kvd, 'ksd': ksd, 'bmap': bmap, 'bias': bias})
        in2.append(m)
    nc2 = _get_nc('p2')
    res2 = run_bass_kernel_spmd(nc2, in2, core_ids=list(range(8)), trace=_tr)
    if res2.exec_time_ns:
        EXEC_NS.append(res2.exec_time_ns)

    out = np.empty((B, L, C), np.float32)
    for ci in range(8):
        b, s = ci // 4, ci % 4
        lo, hi = s * R, min((s + 1) * R, L)
        out[b, lo:hi] = res2.results[ci]['outT'][:, :hi - lo].T
    return out.reshape(B * V, HW, C)
